# revision 21
# baseline (speedup 1.0000x reference)
"""ChebConv (K=3) forward as a distributed Bass/Tile kernel on 8 trn2 NeuronCores.

Sharding: vertices V are sharded across the 8 cores (graph parallel).
  x0 = [x[0] | x[1]]            # [V, 128], feature col = b*64 + fin
  x1 = L @ x0                   # SpMM (COO, edge-parallel)
  out = x0 (W0-W2) + x1 W1 + (L x1) (2 W2) + bias   # x2 folded into weights

Each core owns a row shard (V/8 rows, 98 blocks of 128). SpMM per core,
per 128-edge tile (PSUM accumulate f32):
  - gpsimd.dma_gather fetches 256B source feature rows (bf16) from a
    flat-indexed table laid out in HALVES (per-core row-halves grouped
    across cores) so the x1 AllGather can be split 2 ways and overlapped.
  - The selector M[e,j] = val[e] * (lrow[e]==j) is HOST-precomputed in FP8
    (e4m3) and streamed via HWDGE sync-DMA; the PE runs mixed-precision
    matmuls (fp8 selector x bf16 gathered rows -> f32 PSUM).
  - spmm1: out_block += M^T @ G (row-major x1); spmm2: out_block^T += G^T @ M
    (transposed (L x1), feeding the channel mix directly).
The channel mix runs transposed (pm^T[fo,v] = sum_k Wk'^T xk^T) per block:
x0^T comes host-transposed, (Lx1)^T from spmm2's PSUM, x1^T via an XBAR
DMA transpose (dma_start_transpose) from the local x1 shard. Bias is fused
into the PSUM eviction on the scalar engine. Output is written transposed
and re-transposed on the host during assembly.
"""

import sys

sys.path.insert(0, "/opt/trn_rl_repo")

import numpy as np
import ml_dtypes

import concourse.bass as bass
import concourse.bacc as bacc
import concourse.mybir as mybir
import concourse.tile as tile
from concourse import bass_utils
from concourse.alu_op_type import AluOpType

P = 128
F32 = mybir.dt.float32
BF16 = mybir.dt.bfloat16
FP8 = mybir.dt.float8e4
I16 = mybir.dt.int16
NPBF16 = ml_dtypes.bfloat16
NPFP8 = ml_dtypes.float8_e4m3fn
NQ = 4  # SWDGE queues (parallel Q7 descriptor generation)

VSH = 12500  # V / ncores
NBLK = 98
VPAD = NBLK * P  # 12544
# Row halves (local padded rows) -> 2 gather tables, 2 split AllGathers.
HLO = [0, 6144]
HSZ = [6144, 6400]
HBLK = [(0, 48), (48, 98)]  # block ranges per half
HSTART = [0, 49152]  # table row starts
VTOT = 100352
# Gather windows (<=32768 table rows each; staggered to minimize tile pad).
CHUNK_LO = [0, 21376, 49152, 77696]
CHUNK_HI = [21376, 49152, 77696, 100352]
CHUNK_HALF = [0, 0, 1, 1]

DEBUG_DUMP = False


def _cdiv(a, b):
    return -(-a // b)


def _half_of_block(b):
    return 0 if b < HBLK[0][1] else 1


# ---------------------------------------------------------------------------
# Host-side: uniform (cross-core) edge structure + per-core content arrays
# ---------------------------------------------------------------------------


class EdgeStructure:
    def __init__(self, V, ncores, sb_blocks, rows, cols):
        assert V % ncores == 0 and V // ncores == VSH
        self.V, self.ncores = V, ncores
        self.vsh = VSH
        self.nblk = NBLK
        self.vpad = VPAD
        self.vtot = VTOT
        nchunks = self.nchunks = 4

        rows = np.asarray(rows, np.int64)
        cols = np.asarray(cols, np.int64)
        c_of = cols // VSH
        loc = cols - c_of * VSH
        half = (loc >= HLO[1]).astype(np.int64)
        flat = np.where(
            half == 0,
            c_of * HSZ[0] + loc,
            HSTART[1] + c_of * HSZ[1] + (loc - HLO[1]),
        )
        r_core = rows // VSH
        r_loc = rows - r_core * VSH
        blk = r_loc // P
        chunk = (
            np.searchsorted(np.array(CHUNK_LO, np.int64), flat, side="right")
            - 1
        )

        # slot order: for sb: for chunk: for block in sb
        sb_arr = blk // sb_blocks
        bi_arr = blk % sb_blocks
        bh_arr = np.minimum(sb_blocks, self.nblk - sb_arr * sb_blocks)
        sid = sb_arr * sb_blocks * nchunks + chunk * bh_arr + bi_arr

        self.sb_blocks = sb_blocks
        self.nsb = _cdiv(self.nblk, sb_blocks)
        order = []
        for sb in range(self.nsb):
            b0 = sb * sb_blocks
            bh = min(sb_blocks, self.nblk - b0)
            for ch in range(nchunks):
                for bi in range(bh):
                    order.append((b0 + bi, ch))
        self.nslots = len(order)
        self.slot_block = np.array([b for b, _ in order], np.int64)
        self.slot_chunk = np.array([c for _, c in order], np.int64)

        counts = np.zeros((ncores, self.nslots), np.int64)
        np.add.at(counts, (r_core, sid), 1)
        T = _cdiv(np.max(counts, axis=0), P)

        # every block needs >=1 tile so its PSUM accumulator gets written
        blk_tiles = np.zeros(self.nblk, np.int64)
        np.add.at(blk_tiles, self.slot_block, T)
        for b in np.nonzero(blk_tiles == 0)[0]:
            sb, bi = b // sb_blocks, b % sb_blocks
            bh = min(sb_blocks, self.nblk - sb * sb_blocks)
            T[sb * sb_blocks * nchunks + 0 * bh + bi] = 1

        self.T = T
        self.slot_tile_base = np.concatenate(([0], np.cumsum(T)))[:-1]
        self.ntiles = int(np.sum(T))
        self.sid_of_edge = sid
        self.flat_of_edge = flat
        self.r_core_of_edge = r_core
        self.lrow_of_edge = (r_loc % P).astype(np.int64)

        # (sb, chunk) -> contiguous tile run
        self.runs = []  # per sb: list of (tile_start, ntiles, chunk)
        s = 0
        for sb in range(self.nsb):
            b0 = sb * sb_blocks
            bh = min(sb_blocks, self.nblk - b0)
            sb_runs = []
            for ch in range(nchunks):
                t0 = int(self.slot_tile_base[s])
                ntr = int(np.sum(T[s : s + bh]))
                if ntr > 0:
                    sb_runs.append((t0, ntr, ch))
                s += bh
            self.runs.append(sb_runs)
        self.max_run_tiles = max(
            nt for sb_runs in self.runs for _, nt, _ in sb_runs
        )

        tile_block = np.empty(self.ntiles, np.int64)
        for s in range(self.nslots):
            t0, ntr = self.slot_tile_base[s], T[s]
            tile_block[t0 : t0 + ntr] = self.slot_block[s]
        self.tile_block = tile_block

    def start_stop(self, sb_order, chunk_order):
        """Per-block first/last executed tile for a given execution order."""
        tile_start = np.zeros(self.ntiles, bool)
        tile_stop = np.zeros(self.ntiles, bool)
        first, last = {}, {}
        for sb in sb_order:
            runs = {ch: (t0, ntr) for (t0, ntr, ch) in self.runs[sb]}
            for ch in chunk_order:
                if ch not in runs:
                    continue
                t0, ntr = runs[ch]
                for t in range(t0, t0 + ntr):
                    b = int(self.tile_block[t])
                    if b not in first:
                        first[b] = t
                    last[b] = t
        for t in first.values():
            tile_start[t] = True
        for t in last.values():
            tile_stop[t] = True
        return tile_start, tile_stop

    def per_core_arrays(self, core, vals):
        """idx (int16 wrapped+replicated) and fp8 M tiles for one core."""
        sel = np.nonzero(self.r_core_of_edge == core)[0]
        sid = self.sid_of_edge[sel]
        o = np.argsort(sid, kind="stable")
        sel, sid = sel[o], sid[o]
        start = np.searchsorted(sid, np.arange(self.nslots))
        rank = np.arange(len(sid)) - start[sid]
        pos = self.slot_tile_base[sid] * P + rank
        n = self.ntiles * P
        idx = np.zeros(n, np.int16)
        idx[pos] = (
            self.flat_of_edge[sel]
            - np.array(CHUNK_LO, np.int64)[self.slot_chunk[sid]]
        ).astype(np.int16)
        idx_w = np.tile(np.ascontiguousarray(idx.reshape(-1, 16).T), (8, 1))
        # M tiles: M[t, e, lrow] = val; stored partition-major [P, nt*P]
        m = np.zeros((self.ntiles, P, P), np.float32)
        m[pos // P, pos % P, self.lrow_of_edge[sel]] = vals[sel]
        mfull = np.ascontiguousarray(
            m.astype(NPFP8).transpose(1, 0, 2).reshape(P, self.ntiles * P)
        )
        return idx_w, mfull


# ---------------------------------------------------------------------------
# Bass program (SPMD: one program, per-core data via in_maps)
# ---------------------------------------------------------------------------


def build_program(es: EdgeStructure):
    nblk, vpad, ncores = es.nblk, es.vpad, es.ncores
    nt, GW, SB = es.ntiles, es.max_run_tiles, es.sb_blocks

    nc = bacc.Bacc(
        "TRN2",
        target_bir_lowering=False,
        debug=False,
        num_devices=ncores,
        num_swdge_queues=NQ,
    )

    x0g = nc.dram_tensor("x0g", [VTOT, P], BF16, kind="ExternalInput")
    x0t = nc.dram_tensor("x0t", [nblk, P, P], BF16, kind="ExternalInput")
    wbd = nc.dram_tensor("wbd", [3, P, P], BF16, kind="ExternalInput")
    biasT = nc.dram_tensor("biasT", [P, 1], F32, kind="ExternalInput")
    tgidx = nc.dram_tensor("tgidx", [P, 8], I16, kind="ExternalInput")
    eidx = nc.dram_tensor("eidx", [P, nt * 8], I16, kind="ExternalInput")
    emt = nc.dram_tensor("emt", [P, nt * P], FP8, kind="ExternalInput")
    outp = nc.dram_tensor("outp", [P, vpad], F32, kind="ExternalOutput")

    x1my = [
        nc.dram_tensor(f"x1myH{h}", [HSZ[h], P], BF16)
        for h in range(2)
    ]
    x1g = [
        nc.dram_tensor(f"x1gH{h}", [ncores * HSZ[h], P], BF16)
        for h in range(2)
    ]
    if DEBUG_DUMP:
        d_lx1 = nc.dram_tensor(
            "d_lx1", [nblk, P, P], BF16, kind="ExternalOutput"
        )
        d_x1t = nc.dram_tensor(
            "d_x1t", [nblk, P, P], BF16, kind="ExternalOutput"
        )
        d_x1my = nc.dram_tensor(
            "d_x1my", [vpad, P], BF16, kind="ExternalOutput"
        )
        d_x1g0 = nc.dram_tensor(
            "d_x1g0", [ncores * HSZ[0], P], BF16, kind="ExternalOutput"
        )
        d_x1g1 = nc.dram_tensor(
            "d_x1g1", [ncores * HSZ[1], P], BF16, kind="ExternalOutput"
        )

    with tile.TileContext(nc) as tc:
        with (
            tc.tile_pool(name="const", bufs=1) as cpool,
            tc.tile_pool(name="gslab", bufs=10) as gpool,
            tc.tile_pool(name="mslab", bufs=10) as mpool,
            tc.tile_pool(name="ivl", bufs=10) as ipool,
            tc.tile_pool(name="xt", bufs=2 * SB) as xpool,
            tc.tile_pool(name="ostage", bufs=2 * SB) as opool,
            tc.tile_pool(name="acc", bufs=2 * SB, space="PSUM") as apool,
            tc.tile_pool(name="pmix", bufs=2, space="PSUM") as pmpool,
        ):
            biasT_s = cpool.tile([P, 1], F32, tag="biasT")
            nc.sync.dma_start(out=biasT_s[:], in_=biasT[:, :])
            tgidx_s = cpool.tile([P, 8], I16, tag="tgidx")
            nc.sync.dma_start(out=tgidx_s[:], in_=tgidx[:, :])
            wbd_s = cpool.tile([P, 3 * P], BF16, tag="wbd")
            for k in range(3):
                nc.sync.dma_start(
                    out=wbd_s[:, k * P : (k + 1) * P], in_=wbd[k, :, :]
                )

            qn = [0]

            def spmm(chunk_src, layout_b, pre_sb, out_cb, post_sb,
                     sb_order, chunk_order):
                tile_start, tile_stop = es.start_stop(sb_order, chunk_order)
                for sb in sb_order:
                    b0 = sb * SB
                    bh = min(SB, nblk - b0)
                    pre = pre_sb(b0, bh) if pre_sb else None
                    psums = {
                        b0 + bi: apool.tile(
                            [P, P], F32, tag="acc", name=f"acc{b0 + bi}"
                        )
                        for bi in range(bh)
                    }
                    runs_d = {
                        ch: (t0, ntr) for (t0, ntr, ch) in es.runs[sb]
                    }
                    for ch in chunk_order:
                        if ch not in runs_d:
                            continue
                        t0, ntr = runs_d[ch]
                        it = ipool.tile([P, GW * 8], I16, tag="idx")
                        nc.sync.dma_start(
                            out=it[:, : ntr * 8],
                            in_=eidx[:, t0 * 8 : (t0 + ntr) * 8],
                        )
                        mt = mpool.tile([P, GW * P], FP8, tag="m")
                        nc.sync.dma_start(
                            out=mt[:, : ntr * P],
                            in_=emt[:, t0 * P : (t0 + ntr) * P],
                        )
                        g = gpool.tile([P, GW * P], BF16, tag="g")
                        nidx = ntr * P
                        nc.gpsimd.dma_gather(
                            out_ap=g[:, :nidx].rearrange(
                                "p (t e) -> p t e", e=P
                            ),
                            in_ap=chunk_src(ch),
                            idxs_ap=it[:, : ntr * 8],
                            num_idxs=nidx,
                            num_idxs_reg=nidx,
                            elem_size=P,
                            single_packet=False,
                            queue_num=qn[0] % NQ,
                        )
                        qn[0] += 1
                        for tt in range(ntr):
                            t = t0 + tt
                            b = int(es.tile_block[t])
                            gt = g[:, tt * P : (tt + 1) * P]
                            mm = mt[:, tt * P : (tt + 1) * P]
                            start = bool(tile_start[t])
                            stop = bool(tile_stop[t])
                            if layout_b:
                                nc.tensor.matmul(
                                    out=psums[b][:], lhsT=gt, rhs=mm,
                                    start=start, stop=stop,
                                )
                            else:
                                nc.tensor.matmul(
                                    out=psums[b][:], lhsT=mm, rhs=gt,
                                    start=start, stop=stop,
                                )
                    for bi in range(bh):
                        out_cb(b0 + bi, psums[b0 + bi], pre)
                    if post_sb:
                        post_sb(sb)

            # ---------------- SpMM 1: x1 = L @ x0 (row-major out) --------
            def cb1(b, ps, _pre):
                xb = opool.tile([P, P], BF16, tag="x1st")
                nc.scalar.copy(out=xb[:], in_=ps[:])
                h = _half_of_block(b)
                r0 = b * P - HLO[h]
                nc.scalar.dma_start(
                    out=x1my[h][r0 : r0 + P, :], in_=xb[:]
                )

            ag_done = [False] * 2

            def issue_ag(h):
                if ag_done[h]:
                    return
                ag_done[h] = True
                nc.gpsimd.collective_compute(
                    "AllGather",
                    AluOpType.bypass,
                    replica_groups=[list(range(ncores))],
                    ins=[x1my[h].ap().opt()],
                    outs=[x1g[h].ap().opt()],
                )

            done_blocks = set()

            def post_sb1(sb):
                for b in range(sb * SB, min((sb + 1) * SB, nblk)):
                    done_blocks.add(b)
                for h in range(2):
                    if all(
                        b in done_blocks
                        for b in range(HBLK[h][0], HBLK[h][1])
                    ):
                        issue_ag(h)

            spmm(
                lambda ch: x0g[CHUNK_LO[ch] : CHUNK_HI[ch], :],
                False,
                None,
                cb1,
                post_sb1,
                list(range(es.nsb)),
                [0, 1, 2, 3],
            )
            for h in range(2):
                issue_ag(h)

            # -------- SpMM 2 (transposed out) + fused channel mix --------
            def pre_sb2(b0, bh):
                pre = {}
                for bi in range(bh):
                    b = b0 + bi
                    x0b = xpool.tile([P, P], BF16, tag="x0b", name=f"x0b{b}")
                    nc.scalar.dma_start(out=x0b[:], in_=x0t[b, :, :])
                    x1b = xpool.tile([P, P], BF16, tag="x1b", name=f"x1b{b}")
                    h = _half_of_block(b)
                    r0 = b * P - HLO[h]
                    nc.scalar.dma_start_transpose(
                        out=x1b[:], in_=x1my[h][r0 : r0 + P, :]
                    )
                    if DEBUG_DUMP:
                        nc.scalar.dma_start(out=d_x1t[b, :, :], in_=x1b[:])
                    pre[b] = (x0b, x1b)
                return pre

            def cb2(b, ps, pre):
                x0b, x1b = pre[b]
                x2b = opool.tile([P, P], BF16, tag="x2b")
                nc.scalar.copy(out=x2b[:], in_=ps[:])
                if DEBUG_DUMP:
                    nc.scalar.dma_start(out=d_lx1[b, :, :], in_=x2b[:])
                pm = pmpool.tile([P, P], F32, tag="pmix")
                for k, xk in enumerate((x0b, x1b, x2b)):
                    nc.tensor.matmul(
                        out=pm[:],
                        lhsT=wbd_s[:, k * P : (k + 1) * P],
                        rhs=xk[:],
                        start=(k == 0),
                        stop=(k == 2),
                    )
                ob = opool.tile([P, P], F32, tag="ob")
                nc.scalar.add(out=ob[:], in_=pm[:], add=biasT_s[:, 0:1])
                nc.scalar.dma_start(
                    out=outp[:, b * P : (b + 1) * P], in_=ob[:]
                )

            def chunk_src2(ch):
                h = CHUNK_HALF[ch]
                lo = CHUNK_LO[ch] - HSTART[h]
                hi = CHUNK_HI[ch] - HSTART[h]
                return x1g[h][lo:hi, :]

            spmm(
                chunk_src2, True, pre_sb2, cb2, None,
                list(range(es.nsb)), [0, 1, 2, 3],
            )

            if DEBUG_DUMP:
                nc.sync.dma_start(
                    out=d_x1my[0 : HSZ[0], :], in_=x1my[0][:, :]
                )
                nc.sync.dma_start(
                    out=d_x1my[HSZ[0] :, :], in_=x1my[1][:, :]
                )
                nc.sync.dma_start(out=d_x1g0[:, :], in_=x1g[0][:, :])
                nc.sync.dma_start(out=d_x1g1[:, :], in_=x1g[1][:, :])

    nc.compile()
    return nc


# ---------------------------------------------------------------------------
# Host driver
# ---------------------------------------------------------------------------


def prepare(x, weight, bias, lap_vals, lap_rows, lap_cols, ncores=8,
            sb_blocks=3):
    x = np.asarray(x, np.float32)
    weight = np.asarray(weight, np.float32)
    bias = np.asarray(bias, np.float32)
    lap_vals = np.asarray(lap_vals, np.float32)
    lap_rows = np.asarray(lap_rows)
    lap_cols = np.asarray(lap_cols)
    B, V, FIN = x.shape
    _, K, FOUT = weight.shape
    assert B == 2 and FIN == 64 and K == 3 and FOUT == 64 and V == 100000

    es = EdgeStructure(V, ncores, sb_blocks, lap_rows, lap_cols)

    x0 = np.concatenate([x[0], x[1]], axis=1)  # [V, 128] f32
    # x0g: half-grouped gather table [VTOT, 128] bf16
    x0g = np.zeros((VTOT, P), NPBF16)
    for h in range(2):
        for o in range(ncores):
            lo = o * VSH + HLO[h]
            hi = min(o * VSH + min(HLO[h] + HSZ[h], VSH), V)
            n = hi - lo
            if n > 0:
                r0 = HSTART[h] + o * HSZ[h]
                x0g[r0 : r0 + n] = x0[lo:hi].astype(NPBF16)

    wk = [weight[:, k, :] for k in range(3)]
    wfold = [wk[0] - wk[2], wk[1], 2.0 * wk[2]]
    wbd = np.zeros((3, P, P), np.float32)
    for k in range(3):
        wbd[k, :64, :64] = wfold[k]
        wbd[k, 64:, 64:] = wfold[k]
    wbd = wbd.astype(NPBF16)
    biasT = np.concatenate([bias, bias]).reshape(P, 1).astype(np.float32)
    tgidx = np.tile(
        np.ascontiguousarray(np.arange(P, dtype=np.int16).reshape(-1, 16).T),
        (8, 1),
    )

    in_maps = []
    for c in range(ncores):
        idx_w, mfull = es.per_core_arrays(c, lap_vals)
        xc = np.zeros((es.vpad, P), np.float32)
        xc[: es.vsh] = x0[c * es.vsh : (c + 1) * es.vsh]
        x0t_c = np.ascontiguousarray(
            xc.reshape(es.nblk, P, P).transpose(0, 2, 1).astype(NPBF16)
        )
        in_maps.append(
            {
                "x0g": x0g,
                "x0t": x0t_c,
                "wbd": wbd,
                "biasT": biasT,
                "tgidx": tgidx,
                "eidx": idx_w,
                "emt": mfull,
            }
        )

    nc = build_program(es)

    def assemble(results):
        out = np.empty((B, V, FOUT), np.float32)
        for c in range(ncores):
            o = np.asarray(results[c]["outp"]).reshape(B, FOUT, es.vpad)
            out[:, c * es.vsh : (c + 1) * es.vsh, :] = o[
                :, :, : es.vsh
            ].transpose(0, 2, 1)
        return out

    return nc, in_maps, assemble, es


def kernel(x, weight, bias, lap_vals, lap_rows, lap_cols):
    nc, in_maps, assemble, es = prepare(
        x, weight, bias, lap_vals, lap_rows, lap_cols
    )
    res = bass_utils.run_bass_kernel_spmd(
        nc, in_maps, core_ids=list(range(es.ncores))
    )
    return assemble(res.results)
